# revision 75
# baseline (speedup 1.0000x reference)
"""Masked attention kernel for Trainium2, 8 NeuronCores.

Problem: q,k,v [32,1024,64] f32, mask [32,1024,1024] bool (True -> -inf),
out = softmax(q@k^T * D^-0.5 masked) @ v.

Sharding: batch*head dim (32) split across 8 cores, 4 heads/core.

Per-core device algorithm (T-layout), ACT-paced pipeline:
  scoresT[t,s] = sum_d k[t,d] q[s,d]  via PE row-group pairs (qkT
      host-duplicated into partitions 64-127, two t-tiles concurrent).
  pT = exp(0.125 * scoresT) on ACT (no row-max: |0.125 s| <= ~6; the
      32 exps at ~1.1us each are the pacing engine).
  mask applied multiplicatively on DVE after exp: pT *= keepT
      (keep = 1-mask, bf16, one batched 2x-rate tensor_mul per pair).
  outT_aug[d,s] = sum_t v_aug[t,d] pT[t,s], v_aug = [v | ones] so row 64
      carries softmax denominators; accumulated in s-halves of 512 so the
      PSUM budget fits 3 score slots (ACT never starves).
  tail: outT+sums transposed together via 65-row PE transposes (the sums
      become column 64 of each transposed block), reciprocal runs on a
      strided [128,8] view, division is one broadcast tensor_mul, output
      DMA'd as bf16.
PSUM budget (8 banks): 3 score slots (6) + o_ps half (1) + f_ps (1).
HAM management: an initial ~3.7us contiguous dummy-matmul streak flips
the PE clock gate to 2.4 GHz before the first real matmul, and one
filler matmul per pair (recycling a drained score slot) keeps the
busy-window saturated so the gate never re-throttles mid-kernel.
AV bursts lag one head; tails are deferred one further pair. All DRAM
tensors host pre-tiled so every DMA is a straight 128-partition
contiguous copy. Host does only layout work (transposes/casts/tiling).
"""

import os
import sys

import numpy as np

for _p in ("/opt/trn_rl_repo", "/opt/pypackages"):
    if _p not in sys.path and os.path.isdir(_p):
        sys.path.append(_p)

import ml_dtypes  # noqa: E402

import concourse.bass as bass  # noqa: E402
import concourse.tile as tile  # noqa: E402
from concourse import mybir  # noqa: E402
from concourse.bass_utils import run_bass_kernel_spmd  # noqa: E402

BH, S, D = 32, 1024, 64
NCORES = 8
HPC = BH // NCORES  # heads per core
NT = S // 128  # 8 tiles of 128 along s/t
F32 = mybir.dt.float32
BF16 = mybir.dt.bfloat16


def _build_program():
    nc = bass.Bass(
        "TRN2",
        target_bir_lowering=False,
        debug=False,
        num_devices=NCORES,
    )
    qkt = nc.dram_tensor("qkt", [HPC, 128, 2 * S], BF16, kind="ExternalInput").ap()
    vaug = nc.dram_tensor("vaug", [HPC, 128, NT * 65], BF16, kind="ExternalInput").ap()
    mtb = nc.dram_tensor("mtb", [HPC, 128, NT * S], BF16, kind="ExternalInput").ap()
    ident = nc.dram_tensor("ident", [65, 65], BF16, kind="ExternalInput").ap()
    outp = nc.dram_tensor("outp", [HPC, 128, NT * D], BF16, kind="ExternalOutput").ap()

    with tile.TileContext(nc) as tc:
        with (
            tc.tile_pool(name="const", bufs=1) as const_pool,
            tc.tile_pool(name="qk", bufs=HPC) as qk_pool,
            tc.tile_pool(name="v", bufs=HPC) as v_pool,
            tc.tile_pool(name="m", bufs=HPC) as m_pool,
            tc.tile_pool(name="p", bufs=2) as p_pool,
            tc.tile_pool(name="ot", bufs=2) as ot_pool,
            tc.tile_pool(name="fin", bufs=2) as fin_pool,
            tc.tile_pool(name="spsum", bufs=1, space="PSUM") as s_pool,
            tc.tile_pool(name="opsum", bufs=1, space="PSUM") as o_pool,
            tc.tile_pool(name="fpsum", bufs=1, space="PSUM") as f_pool,
        ):
            ident_sb = const_pool.tile([65, 65], BF16)
            warm_sb = const_pool.tile([1, 1], F32)
            nc.vector.memset(warm_sb[:], 0.0)
            warm_out = const_pool.tile([1, 1], F32, tag="warmo")
            nc.scalar.activation(
                out=warm_out[:],
                in_=warm_sb[:],
                func=mybir.ActivationFunctionType.Exp,
            )
            # HAM warmup: a solid PE busy-streak from t=0 flips the clock
            # gate to 8/8 (2.4 GHz) ~3.4us in, so the first heads don't run
            # at half clock. N=512 matmuls on a memset tile give real
            # occupancy; extra fillers are woven between the early pairs.
            warm_rhs = const_pool.tile([128, 512], BF16, tag="warmr")
            nc.vector.memset(warm_rhs[:], 0.0)
            warm_lhs = const_pool.tile([128, 128], BF16, tag="warml")
            nc.vector.memset(warm_lhs[:], 0.0)
            def emit_warm(n, target=None):
                # fillers recycle an already-drained score tile (no fresh
                # pool alloc -- that would starve the ACT of score slots)
                if target is None:
                    w_ps = s_pool.tile([128, 512], F32, name="warm_ps", tag="sc")
                else:
                    w_ps = target
                for i in range(n):
                    nc.tensor.matmul(
                        out=w_ps[:],
                        lhsT=warm_lhs[:],
                        rhs=warm_rhs[:],
                        start=(i == 0),
                        stop=(i == n - 1),
                    )

            # a contiguous cold busy-streak (continued seamlessly by the
            # first QK pairs) trips the HAM SHORT window deterministically,
            # so the first real matmuls run warm
            emit_warm(6)

            # one manually-slotted score buffer instead of a 3-tile pool
            # rotation: when a pair's two slots are memory-adjacent (11 of
            # 16 pairs), ONE N=2048 activation drains both tiles, halving
            # the per-instruction ACT overhead (~290ns each) on the pacing
            # engine. Byte-range dependency tracking supplies the same
            # WAR/RAW ordering the rotation did.
            sc_mega = s_pool.tile([128, 3 * S], F32, name="sc_mega", tag="sc")

            qk_tiles, v_tiles, m_tiles = [], [], []
            for h in range(HPC):
                qk_sb = qk_pool.tile([128, 2 * S], BF16)
                m_sb = m_pool.tile([128, NT * S], BF16)
                v_sb = v_pool.tile([128, NT * 65], BF16)
                if h == 0:
                    nc.sync.dma_start(qk_sb[:, : S + 256], qkt[h][:, : S + 256])
                    nc.sync.dma_start(qk_sb[:, S + 256 :], qkt[h][:, S + 256 :])
                    nc.sync.dma_start(v_sb[:], vaug[h])
                    nc.sync.dma_start(m_sb[:, : 2 * S], mtb[h][:, : 2 * S])
                    nc.sync.dma_start(m_sb[:, 2 * S :], mtb[h][:, 2 * S :])
                else:
                    nc.sync.dma_start(qk_sb[:], qkt[h])
                    nc.sync.dma_start(v_sb[:], vaug[h])
                    nc.sync.dma_start(m_sb[:], mtb[h])
                qk_tiles.append(qk_sb)
                v_tiles.append(v_sb)
                m_tiles.append(m_sb)
                if h == 0:
                    # ident (8KB) is first needed by the tail of head 0;
                    # issuing it here keeps it off head 0's critical path
                    # while landing well before ~20us
                    nc.sync.dma_start(ident_sb[:], ident[:])

            p_tiles = {}
            pair_counter = {"g": 0}

            def emit_pair(h, pr):
                """QK + exp + multiplicative mask for t-tiles (2*pr, 2*pr+1).
                The two tiles' matmuls interleave so their disjoint PE row
                groups stream concurrently."""
                qk_sb, m_sb = qk_tiles[h], m_tiles[h]
                p_sb = p_tiles[h]
                g = pair_counter["g"]
                pair_counter["g"] += 1
                slots = ((2 * g) % 3, (2 * g + 1) % 3)
                for n in range(2):
                    for i in (0, 1):
                        t = 2 * pr + i
                        base = slots[i] * S
                        rows = slice(64 * i, 64 * i + 64)
                        kslc = slice(S + t * 128, S + (t + 1) * 128)
                        nc.tensor.matmul(
                            out=sc_mega[:, base + n * 512 : base + (n + 1) * 512],
                            lhsT=qk_sb[rows, kslc],
                            rhs=qk_sb[rows, n * 512 : (n + 1) * 512],
                            start=True,
                            stop=True,
                        )
                tlo = 2 * pr
                if slots[1] == slots[0] + 1:
                    # adjacent slots: one batched exp over both tiles
                    nc.scalar.activation(
                        out=p_sb[:, tlo * S : (tlo + 2) * S],
                        in_=sc_mega[:, slots[0] * S : (slots[0] + 2) * S],
                        func=mybir.ActivationFunctionType.Exp,
                        scale=0.125,
                    )
                else:
                    for i in (0, 1):
                        nc.scalar.activation(
                            out=p_sb[:, (tlo + i) * S : (tlo + i + 1) * S],
                            in_=sc_mega[:, slots[i] * S : (slots[i] + 1) * S],
                            func=mybir.ActivationFunctionType.Exp,
                            scale=0.125,
                        )
                scs = [(tlo, sc_mega[:, slots[0] * S : slots[0] * S + 512])]
                if h == HPC - 1:
                    # per-tile mults through the last head: its AV matmuls
                    # wait only on their own tile's mult, and the DVE queue
                    # never builds an endgame backlog
                    for t in (tlo, tlo + 1):
                        psl = slice(t * S, (t + 1) * S)
                        nc.vector.tensor_mul(
                            out=p_sb[:, psl],
                            in0=p_sb[:, psl],
                            in1=m_sb[:, psl],
                        )
                    return scs
                # one batched mask multiply for the pair (fewer DVE ops and
                # semaphore edges; 2x mode needs bf16 step-1 operands)
                psl = slice(2 * pr * S, (2 * pr + 2) * S)
                nc.vector.tensor_mul(
                    out=p_sb[:, psl],
                    in0=p_sb[:, psl],
                    in1=m_sb[:, psl],
                )
                return scs

            av_state = {}

            def emit_av_half(h, half, from_sc=False):
                """AV for s-columns [half*512, half*512+512) of head h: all 8
                t-tiles accumulate into a 1-bank o_ps, drained to ot_sb."""
                v_sb = v_tiles[h]
                p_sb = p_tiles[h]
                if from_sc:
                    # endgame: score slots are free, borrow one so both
                    # halves run back-to-back without waiting on the drain
                    o_ps = s_pool.tile([65, 512], F32, name="o_sc", tag="sc")
                else:
                    o_ps = o_pool.tile([65, 512], F32, name="o_ps")
                base = half * 512
                for t in range(NT):
                    nc.tensor.matmul(
                        out=o_ps[:],
                        lhsT=v_sb[:, t * 65 : (t + 1) * 65],
                        rhs=p_sb[:, t * S + base : t * S + base + 512],
                        start=(t == 0),
                        stop=(t == NT - 1),
                    )
                if half == 0:
                    ot_sb = ot_pool.tile([65, S], BF16, name="ot_sb")
                    av_state[h] = ot_sb
                else:
                    ot_sb = av_state[h]
                nc.vector.tensor_copy(ot_sb[:, base : base + 512], o_ps[:])

            def emit_tail(h):
                """Transpose outT(+sums) back to [s,d], divide, DMA out.
                66-wide blocks keep each bf16 transpose output 4B-aligned."""
                ot_sb = av_state.pop(h)
                f_ps = f_pool.tile([128, NT * 66], BF16, name="f_ps")
                for j in range(NT):
                    nc.tensor.transpose(
                        out=f_ps[:, j * 66 : j * 66 + 65],
                        in_=ot_sb[:, j * 128 : (j + 1) * 128],
                        identity=ident_sb[:],
                    )
                f3 = f_ps[:].rearrange("p (j c) -> p j c", j=NT)
                r_sb = fin_pool.tile([128, NT], F32, tag="rsb")
                nc.vector.reciprocal(r_sb[:], f3[:, :, 64])
                out_sb = fin_pool.tile([128, NT * D], BF16, tag="osb")
                nc.vector.tensor_mul(
                    out=out_sb[:].rearrange("p (j d) -> p j d", j=NT),
                    in0=f3[:, :, 0:64],
                    in1=r_sb[:, :, None].to_broadcast((128, NT, D)),
                )
                nc.sync.dma_start(outp[h], out_sb[:])

            # AV halves lag one head behind so the PE has dense work while
            # ACT drains the current head's score slots; tails are deferred
            # one further pair so the PE never waits on the DVE ot-copy.
            prev_sc = None
            for h in range(HPC):
                p_tiles[h] = p_pool.tile([128, NT * S], BF16, name="p_sb", tag="p")
                for pr in range(4):
                    scs = emit_pair(h, pr)
                    # densify the PE stream: the HAM clock gate re-throttles
                    # to 1.2 GHz whenever a 4096-cycle window sees the PE
                    # idle, and a cold PE is a stable attractor (slower MMs
                    # -> more idle). Cheap filler matmuls pin it warm.
                    if prev_sc is not None:
                        # heads 0-1 run before the AV pipeline fills; denser
                        # fillers there prevent the mid-body cold dip
                        emit_warm(2 if h <= 1 else 1, target=prev_sc)
                    prev_sc = scs[0][1]
                    if pr == 0 and h >= 2:
                        emit_tail(h - 2)
                    elif pr == 1 and h > 0:
                        emit_av_half(h - 1, 0)
                    elif pr == 2 and h > 0:
                        emit_av_half(h - 1, 1)
                p_tiles.pop(h - 2, None)
            emit_tail(HPC - 2)
            emit_av_half(HPC - 1, 0)
            emit_av_half(HPC - 1, 1, from_sc=True)
            emit_tail(HPC - 1)

    if os.environ.get("KERNEL_DEDUPE_LDW", "0") == "1":
        _dedupe_ldweights(nc)
    _split_multi_waits(nc)
    return nc


def _dedupe_ldweights(nc):
    """Bass emits one InstLdweights per matmul; the PE keeps its weight state
    between matmuls, so a reload of the exact same weights AP with only
    matmuls/semaphores in between is pure overhead (~P/1.2 ns each). Drop the
    repeats, preserving their sync conditions via bare EventSemaphores. Data
    hazards stay tracked: the InstMatmult itself carries the weights AP read,
    so the tile framework's semaphore graph is unaffected."""
    for bb in nc.bb_map.values():
        insts = bb.bb.instructions
        new_list = []
        last_key = None
        for inst in insts:
            tn = type(inst).__name__
            eng = getattr(inst, "engine", None)
            if eng != mybir.EngineType.PE:
                new_list.append(inst)
                continue
            if tn == "InstLdweights":
                key = (repr(inst.ins[0]), bool(inst.is_transpose))
                if key == last_key:
                    si = getattr(inst, "sync_info", None)
                    if si is not None and (si.on_wait or si.on_update):
                        new_list.append(
                            mybir.InstEventSemaphore(
                                name=nc.get_next_instruction_name(),
                                ins=[],
                                outs=[],
                                engine=inst.engine,
                                sync_info=si,
                            )
                        )
                    continue
                last_key = key
            elif tn == "InstMatmult":
                if getattr(inst, "is_transpose", False):
                    last_key = None
            elif tn != "InstEventSemaphore":
                last_key = None
            new_list.append(inst)
        insts[:] = new_list


def _split_multi_waits(nc):
    """Walrus's S3_LW codegen can't take >1 sync-wait condition on a Matmult;
    hoist extras into standalone EventSemaphore instructions (same semantics:
    the engine queue stalls on them in program order, like raw-bass wait_ge)."""
    for bb in nc.bb_map.values():
        insts = bb.bb.instructions
        new_list = []
        for inst in insts:
            si = getattr(inst, "sync_info", None)
            if (
                si is not None
                and si.on_wait
                and len(si.on_wait) > 1
            ):
                extra = si.on_wait[:-1]
                keep = si.on_wait[-1:]
                for cond in extra:
                    new_list.append(
                        mybir.InstEventSemaphore(
                            name=nc.get_next_instruction_name(),
                            ins=[],
                            outs=[],
                            engine=inst.engine,
                            sync_info=mybir.SyncInfo(on_wait=[cond], on_update=[]),
                        )
                    )
                si.on_wait = keep
            new_list.append(inst)
        insts[:] = new_list


import concourse.bass_utils as _bu

_orig_run_command = _bu.run_command


# note: --enable-ldw-opt=true is unusable here -- walrus rejects the
# standalone InstLdweights that bass emits for every matmul.

_NC_CACHE = None


def _get_nc():
    global _NC_CACHE
    if _NC_CACHE is None:
        _NC_CACHE = _build_program()
    return _NC_CACHE


def _make_in_maps(q, k, v, mask):
    q = np.ascontiguousarray(np.asarray(q, dtype=np.float32))
    k = np.ascontiguousarray(np.asarray(k, dtype=np.float32))
    v = np.ascontiguousarray(np.asarray(v, dtype=np.float32))
    mask = np.asarray(mask)
    ident_np = np.eye(65, dtype=ml_dtypes.bfloat16)
    ones_col = np.ones((HPC, S, 1), dtype=np.float32)
    in_maps = []
    for c in range(NCORES):
        sl = slice(c * HPC, (c + 1) * HPC)
        qT = q[sl].transpose(0, 2, 1)  # [HPC, 64, S]
        kT = k[sl].transpose(0, 2, 1)
        qk1 = np.concatenate([qT, kT], axis=2)  # [HPC, 64, 2S]
        qkt_np = np.ascontiguousarray(
            np.concatenate([qk1, qk1], axis=1)
        ).astype(ml_dtypes.bfloat16)  # rows duplicated for PE row-group packing
        va = np.concatenate([v[sl], ones_col], axis=2)  # [HPC, S, 65]
        vaug_np = np.ascontiguousarray(
            va.reshape(HPC, NT, 128, 65).transpose(0, 2, 1, 3).reshape(HPC, 128, NT * 65)
        ).astype(ml_dtypes.bfloat16)
        # multiplicative keep-mask (True in the input means "drop")
        mT = (~mask[sl]).transpose(0, 2, 1).astype(np.float32)  # [HPC, t=S, s=S]
        mtb_np = np.ascontiguousarray(
            mT.reshape(HPC, NT, 128, S).transpose(0, 2, 1, 3).reshape(HPC, 128, NT * S)
        ).astype(ml_dtypes.bfloat16)
        in_maps.append(
            {
                "qkt": qkt_np,
                "vaug": vaug_np,
                "mtb": mtb_np,
                "ident": ident_np,
            }
        )
    return in_maps


def _gather(results):
    outs = []
    for c in range(NCORES):
        o = np.asarray(results[c]["outp"]).astype(np.float32)  # [HPC,128,NT*D]
        o = o.reshape(HPC, 128, NT, D).transpose(0, 2, 1, 3).reshape(HPC, S, D)
        outs.append(o)
    return np.ascontiguousarray(np.concatenate(outs, axis=0))


def _install_profile_shim():
    """The agent image's antenv lacks axon_hooks; recreate it from the boot
    module's ctypes implementation so trace=True can capture NTFF profiles."""
    import types

    if "antenv.axon_hooks" in sys.modules:
        return
    try:
        from trn_agent_boot.trn_boot import _ntff_profile_via_ctypes

        hook = _ntff_profile_via_ctypes("/opt/axon/libaxon_pjrt.so")
        mod = types.ModuleType("antenv.axon_hooks")
        mod.get_axon_ntff_profile_hook = lambda: hook
        mod.set_axon_ntff_profile_hook = lambda h: None
        sys.modules["antenv.axon_hooks"] = mod
        # don't try to copy artifacts to a remote bucket from the sandbox
        import concourse.bass_utils as _bu

        _bu.upload_artifacts = lambda tmpdir: tmpdir
    except Exception as e:  # profiling is best-effort
        print(f"profile shim unavailable: {e}", file=sys.stderr)


def run(q, k, v, mask, trace=False, **kw):
    nc = _get_nc()
    if trace:
        _install_profile_shim()
    in_maps = _make_in_maps(q, k, v, mask)
    res = run_bass_kernel_spmd(nc, in_maps, list(range(NCORES)), trace=trace, **kw)
    return _gather(res.results), res


def kernel(q, k, v, mask):
    out, _ = run(q, k, v, mask)
    return out


# revision 76
# speedup vs baseline: 1.3996x; 1.3996x over previous
"""Masked attention kernel for Trainium2, 8 NeuronCores.

Problem: q,k,v [32,1024,64] f32, mask [32,1024,1024] bool (True -> -inf),
out = softmax(q@k^T * D^-0.5 masked) @ v.

Sharding: batch*head dim (32) split across 8 cores, 4 heads/core.

Per-core device algorithm (T-layout), ACT-paced pipeline:
  scoresT[t,s] = sum_d k[t,d] q[s,d]  via PE row-group pairs (qkT
      host-duplicated into partitions 64-127, two t-tiles concurrent).
  pT = exp(0.125 * scoresT) on ACT (no row-max: |0.125 s| <= ~6; the
      32 exps at ~1.1us each are the pacing engine).
  mask applied multiplicatively on DVE after exp: pT *= keepT
      (keep = 1-mask, bf16, one batched 2x-rate tensor_mul per pair).
  outT_aug[d,s] = sum_t v_aug[t,d] pT[t,s], v_aug = [v | ones] so row 64
      carries softmax denominators; accumulated in s-halves of 512 so the
      PSUM budget fits 3 score slots (ACT never starves).
  tail: outT+sums transposed together via 65-row PE transposes (the sums
      become column 64 of each transposed block), reciprocal runs on a
      strided [128,8] view, division is one broadcast tensor_mul, output
      DMA'd as bf16.
PSUM budget (8 banks): 3 score slots (6) + o_ps half (1) + f_ps (1).
HAM management: an initial ~3.7us contiguous dummy-matmul streak flips
the PE clock gate to 2.4 GHz before the first real matmul, and one
filler matmul per pair (recycling a drained score slot) keeps the
busy-window saturated so the gate never re-throttles mid-kernel.
AV bursts lag one head; tails are deferred one further pair. All DRAM
tensors host pre-tiled so every DMA is a straight 128-partition
contiguous copy. Host does only layout work (transposes/casts/tiling).
"""

import os
import sys

import numpy as np

for _p in ("/opt/trn_rl_repo", "/opt/pypackages"):
    if _p not in sys.path and os.path.isdir(_p):
        sys.path.append(_p)

import ml_dtypes  # noqa: E402

import concourse.bass as bass  # noqa: E402
import concourse.tile as tile  # noqa: E402
from concourse import mybir  # noqa: E402
from concourse.bass_utils import run_bass_kernel_spmd  # noqa: E402

BH, S, D = 32, 1024, 64
NCORES = 8
HPC = BH // NCORES  # heads per core
NT = S // 128  # 8 tiles of 128 along s/t
F32 = mybir.dt.float32
BF16 = mybir.dt.bfloat16


def _build_program():
    nc = bass.Bass(
        "TRN2",
        target_bir_lowering=False,
        debug=False,
        num_devices=NCORES,
    )
    qkt = nc.dram_tensor("qkt", [HPC, 128, 2 * S], BF16, kind="ExternalInput").ap()
    vaug = nc.dram_tensor("vaug", [HPC, 128, NT * 65], BF16, kind="ExternalInput").ap()
    mtb = nc.dram_tensor("mtb", [HPC, 128, NT * S], BF16, kind="ExternalInput").ap()
    ident = nc.dram_tensor("ident", [65, 65], BF16, kind="ExternalInput").ap()
    outp = nc.dram_tensor("outp", [HPC, 128, NT * D], BF16, kind="ExternalOutput").ap()

    with tile.TileContext(nc) as tc:
        with (
            tc.tile_pool(name="const", bufs=1) as const_pool,
            tc.tile_pool(name="qk", bufs=HPC) as qk_pool,
            tc.tile_pool(name="v", bufs=HPC) as v_pool,
            tc.tile_pool(name="m", bufs=HPC) as m_pool,
            tc.tile_pool(name="p", bufs=2) as p_pool,
            tc.tile_pool(name="ot", bufs=2) as ot_pool,
            tc.tile_pool(name="fin", bufs=2) as fin_pool,
            tc.tile_pool(name="spsum", bufs=3, space="PSUM") as s_pool,
            tc.tile_pool(name="opsum", bufs=1, space="PSUM") as o_pool,
            tc.tile_pool(name="fpsum", bufs=1, space="PSUM") as f_pool,
        ):
            ident_sb = const_pool.tile([65, 65], BF16)
            warm_sb = const_pool.tile([1, 1], F32)
            nc.vector.memset(warm_sb[:], 0.0)
            warm_out = const_pool.tile([1, 1], F32, tag="warmo")
            nc.scalar.activation(
                out=warm_out[:],
                in_=warm_sb[:],
                func=mybir.ActivationFunctionType.Exp,
            )
            # HAM warmup: a solid PE busy-streak from t=0 flips the clock
            # gate to 8/8 (2.4 GHz) ~3.4us in, so the first heads don't run
            # at half clock. N=512 matmuls on a memset tile give real
            # occupancy; extra fillers are woven between the early pairs.
            warm_rhs = const_pool.tile([128, 512], BF16, tag="warmr")
            nc.vector.memset(warm_rhs[:], 0.0)
            warm_lhs = const_pool.tile([128, 128], BF16, tag="warml")
            nc.vector.memset(warm_lhs[:], 0.0)
            def emit_warm(n, target=None):
                # fillers recycle an already-drained score tile (no fresh
                # pool alloc -- that would starve the ACT of score slots)
                if target is None:
                    w_ps = s_pool.tile([128, 512], F32, name="warm_ps", tag="sc")
                else:
                    w_ps = target[:, 0:512]
                for i in range(n):
                    nc.tensor.matmul(
                        out=w_ps[:],
                        lhsT=warm_lhs[:],
                        rhs=warm_rhs[:],
                        start=(i == 0),
                        stop=(i == n - 1),
                    )

            # a contiguous cold busy-streak (continued seamlessly by the
            # first QK pairs) trips the HAM SHORT window deterministically,
            # so the first real matmuls run warm
            emit_warm(6)

            qk_tiles, v_tiles, m_tiles = [], [], []
            for h in range(HPC):
                qk_sb = qk_pool.tile([128, 2 * S], BF16)
                m_sb = m_pool.tile([128, NT * S], BF16)
                v_sb = v_pool.tile([128, NT * 65], BF16)
                if h == 0:
                    nc.sync.dma_start(qk_sb[:, : S + 256], qkt[h][:, : S + 256])
                    nc.sync.dma_start(qk_sb[:, S + 256 :], qkt[h][:, S + 256 :])
                    nc.sync.dma_start(v_sb[:], vaug[h])
                    nc.sync.dma_start(m_sb[:, : 2 * S], mtb[h][:, : 2 * S])
                    nc.sync.dma_start(m_sb[:, 2 * S :], mtb[h][:, 2 * S :])
                else:
                    nc.sync.dma_start(qk_sb[:], qkt[h])
                    nc.sync.dma_start(v_sb[:], vaug[h])
                    nc.sync.dma_start(m_sb[:], mtb[h])
                qk_tiles.append(qk_sb)
                v_tiles.append(v_sb)
                m_tiles.append(m_sb)
                if h == 0:
                    # ident (8KB) is first needed by the tail of head 0;
                    # issuing it here keeps it off head 0's critical path
                    # while landing well before ~20us
                    nc.sync.dma_start(ident_sb[:], ident[:])

            p_tiles = {}

            def emit_pair(h, pr):
                """QK + exp + multiplicative mask for t-tiles (2*pr, 2*pr+1).
                The two tiles' matmuls interleave so their disjoint PE row
                groups stream concurrently."""
                qk_sb, m_sb = qk_tiles[h], m_tiles[h]
                p_sb = p_tiles[h]
                scs = []
                for i in (0, 1):
                    t = 2 * pr + i
                    sc = s_pool.tile([128, S], F32, tag="sc")
                    scs.append((t, sc))
                for n in range(2):
                    sl = slice(n * 512, (n + 1) * 512)
                    for i in (0, 1):
                        t, sc = scs[i]
                        rows = slice(64 * i, 64 * i + 64)
                        kslc = slice(S + t * 128, S + (t + 1) * 128)
                        nc.tensor.matmul(
                            out=sc[:, sl],
                            lhsT=qk_sb[rows, kslc],
                            rhs=qk_sb[rows, sl],
                            start=True,
                            stop=True,
                        )
                for t, sc in scs:
                    psl = slice(t * S, (t + 1) * S)
                    nc.scalar.activation(
                        out=p_sb[:, psl],
                        in_=sc[:],
                        func=mybir.ActivationFunctionType.Exp,
                        scale=0.125,
                    )
                    if h == HPC - 1:
                        # per-tile mults through the last head: its AV
                        # matmuls wait only on their own tile's mult, and
                        # the DVE queue never builds an endgame backlog
                        nc.vector.tensor_mul(
                            out=p_sb[:, psl],
                            in0=p_sb[:, psl],
                            in1=m_sb[:, psl],
                        )
                if h == HPC - 1:
                    return scs
                # one batched mask multiply for the pair (fewer DVE ops and
                # semaphore edges; 2x mode needs bf16 step-1 operands)
                psl = slice(2 * pr * S, (2 * pr + 2) * S)
                nc.vector.tensor_mul(
                    out=p_sb[:, psl],
                    in0=p_sb[:, psl],
                    in1=m_sb[:, psl],
                )
                return scs

            av_state = {}

            def emit_av_half(h, half, from_sc=False):
                """AV for s-columns [half*512, half*512+512) of head h: all 8
                t-tiles accumulate into a 1-bank o_ps, drained to ot_sb."""
                v_sb = v_tiles[h]
                p_sb = p_tiles[h]
                if from_sc:
                    # endgame: score slots are free, borrow one so both
                    # halves run back-to-back without waiting on the drain
                    o_ps = s_pool.tile([65, 512], F32, name="o_sc", tag="sc")
                else:
                    o_ps = o_pool.tile([65, 512], F32, name="o_ps")
                base = half * 512
                for t in range(NT):
                    nc.tensor.matmul(
                        out=o_ps[:],
                        lhsT=v_sb[:, t * 65 : (t + 1) * 65],
                        rhs=p_sb[:, t * S + base : t * S + base + 512],
                        start=(t == 0),
                        stop=(t == NT - 1),
                    )
                if half == 0:
                    ot_sb = ot_pool.tile([65, S], BF16, name="ot_sb")
                    av_state[h] = ot_sb
                else:
                    ot_sb = av_state[h]
                nc.vector.tensor_copy(ot_sb[:, base : base + 512], o_ps[:])

            def emit_tail(h):
                """Transpose outT(+sums) back to [s,d], divide, DMA out.
                66-wide blocks keep each bf16 transpose output 4B-aligned."""
                ot_sb = av_state.pop(h)
                f_ps = f_pool.tile([128, NT * 66], BF16, name="f_ps")
                for j in range(NT):
                    nc.tensor.transpose(
                        out=f_ps[:, j * 66 : j * 66 + 65],
                        in_=ot_sb[:, j * 128 : (j + 1) * 128],
                        identity=ident_sb[:],
                    )
                f3 = f_ps[:].rearrange("p (j c) -> p j c", j=NT)
                r_sb = fin_pool.tile([128, NT], F32, tag="rsb")
                nc.vector.reciprocal(r_sb[:], f3[:, :, 64])
                out_sb = fin_pool.tile([128, NT * D], BF16, tag="osb")
                nc.vector.tensor_mul(
                    out=out_sb[:].rearrange("p (j d) -> p j d", j=NT),
                    in0=f3[:, :, 0:64],
                    in1=r_sb[:, :, None].to_broadcast((128, NT, D)),
                )
                nc.sync.dma_start(outp[h], out_sb[:])

            # AV halves lag one head behind so the PE has dense work while
            # ACT drains the current head's score slots; tails are deferred
            # one further pair so the PE never waits on the DVE ot-copy.
            prev_sc = None
            for h in range(HPC):
                p_tiles[h] = p_pool.tile([128, NT * S], BF16, name="p_sb", tag="p")
                for pr in range(4):
                    scs = emit_pair(h, pr)
                    # densify the PE stream: the HAM clock gate re-throttles
                    # to 1.2 GHz whenever a 4096-cycle window sees the PE
                    # idle, and a cold PE is a stable attractor (slower MMs
                    # -> more idle). Cheap filler matmuls pin it warm.
                    if prev_sc is not None:
                        # heads 0-1 run before the AV pipeline fills; denser
                        # fillers there prevent the mid-body cold dip
                        emit_warm(2 if h <= 1 else 1, target=prev_sc)
                    prev_sc = scs[0][1]
                    if pr == 0 and h >= 2:
                        emit_tail(h - 2)
                    elif pr == 1 and h > 0:
                        emit_av_half(h - 1, 0)
                    elif pr == 2 and h > 0:
                        emit_av_half(h - 1, 1)
                p_tiles.pop(h - 2, None)
            emit_tail(HPC - 2)
            emit_av_half(HPC - 1, 0)
            emit_av_half(HPC - 1, 1, from_sc=True)
            emit_tail(HPC - 1)

    if os.environ.get("KERNEL_DEDUPE_LDW", "0") == "1":
        _dedupe_ldweights(nc)
    _split_multi_waits(nc)
    return nc


def _dedupe_ldweights(nc):
    """Bass emits one InstLdweights per matmul; the PE keeps its weight state
    between matmuls, so a reload of the exact same weights AP with only
    matmuls/semaphores in between is pure overhead (~P/1.2 ns each). Drop the
    repeats, preserving their sync conditions via bare EventSemaphores. Data
    hazards stay tracked: the InstMatmult itself carries the weights AP read,
    so the tile framework's semaphore graph is unaffected."""
    for bb in nc.bb_map.values():
        insts = bb.bb.instructions
        new_list = []
        last_key = None
        for inst in insts:
            tn = type(inst).__name__
            eng = getattr(inst, "engine", None)
            if eng != mybir.EngineType.PE:
                new_list.append(inst)
                continue
            if tn == "InstLdweights":
                key = (repr(inst.ins[0]), bool(inst.is_transpose))
                if key == last_key:
                    si = getattr(inst, "sync_info", None)
                    if si is not None and (si.on_wait or si.on_update):
                        new_list.append(
                            mybir.InstEventSemaphore(
                                name=nc.get_next_instruction_name(),
                                ins=[],
                                outs=[],
                                engine=inst.engine,
                                sync_info=si,
                            )
                        )
                    continue
                last_key = key
            elif tn == "InstMatmult":
                if getattr(inst, "is_transpose", False):
                    last_key = None
            elif tn != "InstEventSemaphore":
                last_key = None
            new_list.append(inst)
        insts[:] = new_list


def _split_multi_waits(nc):
    """Walrus's S3_LW codegen can't take >1 sync-wait condition on a Matmult;
    hoist extras into standalone EventSemaphore instructions (same semantics:
    the engine queue stalls on them in program order, like raw-bass wait_ge)."""
    for bb in nc.bb_map.values():
        insts = bb.bb.instructions
        new_list = []
        for inst in insts:
            si = getattr(inst, "sync_info", None)
            if (
                si is not None
                and si.on_wait
                and len(si.on_wait) > 1
            ):
                extra = si.on_wait[:-1]
                keep = si.on_wait[-1:]
                for cond in extra:
                    new_list.append(
                        mybir.InstEventSemaphore(
                            name=nc.get_next_instruction_name(),
                            ins=[],
                            outs=[],
                            engine=inst.engine,
                            sync_info=mybir.SyncInfo(on_wait=[cond], on_update=[]),
                        )
                    )
                si.on_wait = keep
            new_list.append(inst)
        insts[:] = new_list


import concourse.bass_utils as _bu

_orig_run_command = _bu.run_command


# note: --enable-ldw-opt=true is unusable here -- walrus rejects the
# standalone InstLdweights that bass emits for every matmul.

_NC_CACHE = None


def _get_nc():
    global _NC_CACHE
    if _NC_CACHE is None:
        _NC_CACHE = _build_program()
    return _NC_CACHE


def _make_in_maps(q, k, v, mask):
    q = np.ascontiguousarray(np.asarray(q, dtype=np.float32))
    k = np.ascontiguousarray(np.asarray(k, dtype=np.float32))
    v = np.ascontiguousarray(np.asarray(v, dtype=np.float32))
    mask = np.asarray(mask)
    ident_np = np.eye(65, dtype=ml_dtypes.bfloat16)
    ones_col = np.ones((HPC, S, 1), dtype=np.float32)
    in_maps = []
    for c in range(NCORES):
        sl = slice(c * HPC, (c + 1) * HPC)
        qT = q[sl].transpose(0, 2, 1)  # [HPC, 64, S]
        kT = k[sl].transpose(0, 2, 1)
        qk1 = np.concatenate([qT, kT], axis=2)  # [HPC, 64, 2S]
        qkt_np = np.ascontiguousarray(
            np.concatenate([qk1, qk1], axis=1)
        ).astype(ml_dtypes.bfloat16)  # rows duplicated for PE row-group packing
        va = np.concatenate([v[sl], ones_col], axis=2)  # [HPC, S, 65]
        vaug_np = np.ascontiguousarray(
            va.reshape(HPC, NT, 128, 65).transpose(0, 2, 1, 3).reshape(HPC, 128, NT * 65)
        ).astype(ml_dtypes.bfloat16)
        # multiplicative keep-mask (True in the input means "drop")
        mT = (~mask[sl]).transpose(0, 2, 1).astype(np.float32)  # [HPC, t=S, s=S]
        mtb_np = np.ascontiguousarray(
            mT.reshape(HPC, NT, 128, S).transpose(0, 2, 1, 3).reshape(HPC, 128, NT * S)
        ).astype(ml_dtypes.bfloat16)
        in_maps.append(
            {
                "qkt": qkt_np,
                "vaug": vaug_np,
                "mtb": mtb_np,
                "ident": ident_np,
            }
        )
    return in_maps


def _gather(results):
    outs = []
    for c in range(NCORES):
        o = np.asarray(results[c]["outp"]).astype(np.float32)  # [HPC,128,NT*D]
        o = o.reshape(HPC, 128, NT, D).transpose(0, 2, 1, 3).reshape(HPC, S, D)
        outs.append(o)
    return np.ascontiguousarray(np.concatenate(outs, axis=0))


def _install_profile_shim():
    """The agent image's antenv lacks axon_hooks; recreate it from the boot
    module's ctypes implementation so trace=True can capture NTFF profiles."""
    import types

    if "antenv.axon_hooks" in sys.modules:
        return
    try:
        from trn_agent_boot.trn_boot import _ntff_profile_via_ctypes

        hook = _ntff_profile_via_ctypes("/opt/axon/libaxon_pjrt.so")
        mod = types.ModuleType("antenv.axon_hooks")
        mod.get_axon_ntff_profile_hook = lambda: hook
        mod.set_axon_ntff_profile_hook = lambda h: None
        sys.modules["antenv.axon_hooks"] = mod
        # don't try to copy artifacts to a remote bucket from the sandbox
        import concourse.bass_utils as _bu

        _bu.upload_artifacts = lambda tmpdir: tmpdir
    except Exception as e:  # profiling is best-effort
        print(f"profile shim unavailable: {e}", file=sys.stderr)


def run(q, k, v, mask, trace=False, **kw):
    nc = _get_nc()
    if trace:
        _install_profile_shim()
    in_maps = _make_in_maps(q, k, v, mask)
    res = run_bass_kernel_spmd(nc, in_maps, list(range(NCORES)), trace=trace, **kw)
    return _gather(res.results), res


def kernel(q, k, v, mask):
    out, _ = run(q, k, v, mask)
    return out
